# revision 1
# baseline (speedup 1.0000x reference)
"""Efficient Channel Attention kernel for 8 Trainium2 NeuronCores.

Problem (B=4, N=4096, C=1024, H=4, HD=256):
    qkv = x @ Wqkv.T                 -> q,k,v per head, [HD, N] layout
    q,k l2-normalized over N; scores = (q*temp) @ k.T   [HD, HD] per (b,h)
    attn = softmax(scores, -1); out = attn @ v; y = out @ Wproj.T + bproj + x

Sharding: core = (batch b, token-half). All channel contractions are local;
the only cross-core coupling is the token(N)-contracted quantities: the raw
Grams k^T q and the q/k squared norms, AllReduce'd (~1MB) within the core
pair sharing a batch. Device layouts are channel-major (transposed); the
host feeds x^T / W^T slices and transposes the returned y^T back.

SBUF/PSUM pool tags are reused across phases (static pool allocation):
  wgt w0-7   : Wqkv^T qk-cols -> Wqkv^T v-cols -> Wproj^T
  xs  xs0-7  : x^T stream (A1) -> x^T stream (A2) -> B scratch / y + residual
  vo  vo0-8  : v chunks -> out^T chunks
  PSUM pA-pD : q/k accum -> sumsq rows -> v accum -> spm/atp -> proj accum
  PSUM pE,pF : Gram accumulators (2 heads each) -> attn@v accum
"""

import numpy as np

B, N, C, H = 4, 4096, 1024, 4
HD = C // H          # 256
NCORES = 8
NL = N // 2          # 2048 tokens per core
KT = C // 128        # 8 channel k-tiles
NT5 = NL // 512      # 4 token super-tiles
EPS = 1e-12

_CACHE = {}


def _build():
    import concourse.mybir as mybir
    import concourse.tile as tile
    from concourse import bacc
    from concourse.masks import make_identity

    f32 = mybir.dt.float32
    f32r = mybir.dt.float32r
    AX = mybir.AxisListType.X
    ADD = mybir.AluOpType.add
    Exp = mybir.ActivationFunctionType.Exp
    Ident = mybir.ActivationFunctionType.Identity

    nc = bacc.Bacc("TRN2", target_bir_lowering=False, debug=False,
                   num_devices=NCORES)

    xT_d = nc.dram_tensor("xT", [C, NL], f32r, kind="ExternalInput").ap()
    wqkT_d = nc.dram_tensor("wqkT", [C, 2 * C], f32r, kind="ExternalInput").ap()
    wvT_d = nc.dram_tensor("wvT", [C, C], f32r, kind="ExternalInput").ap()
    wpT_d = nc.dram_tensor("wpT", [C, C], f32r, kind="ExternalInput").ap()
    bias_d = nc.dram_tensor("bias", [128, KT], f32, kind="ExternalInput").ap()
    tmpv_d = nc.dram_tensor("tmpv", [128, KT], f32, kind="ExternalInput").ap()
    xrT_d = nc.dram_tensor("xrT", [C, NL], f32r, kind="ExternalInput").ap()
    yT_d = nc.dram_tensor("yT", [C, NL], f32, kind="ExternalOutput").ap()

    with tile.TileContext(nc) as tc:
        with (
            tc.tile_pool(name="const", bufs=1) as constp,
            tc.tile_pool(name="wgt", bufs=1) as wgtp,
            tc.tile_pool(name="xs", bufs=1) as xsp,
            tc.tile_pool(name="vo", bufs=1) as vop,
            tc.tile_pool(name="wrk", bufs=1) as wrk,
            tc.tile_pool(name="ps1", bufs=1, space="PSUM") as ps1,
            tc.tile_pool(name="ps2", bufs=1, space="PSUM") as ps2,
            tc.tile_pool(name="dram", bufs=1, space="DRAM") as dramp,
        ):
            P1 = ["pA", "pB", "pC", "pD"]  # 1-bank rotating psum tags

            # ---------------- constants ----------------
            ident = constp.tile([128, 128], f32, name="ident")
            make_identity(nc, ident[:])
            bias_sb = constp.tile([128, KT], f32, name="bias_sb")
            nc.sync.dma_start(bias_sb[:], bias_d[:])
            tmpv_sb = constp.tile([128, KT], f32, name="tmpv_sb")
            nc.sync.dma_start(tmpv_sb[:], tmpv_d[:])
            ones_sb = constp.tile([128, 1], f32, name="ones_sb")
            nc.vector.memset(ones_sb[:], 1.0)

            # first token super-tile of x^T, loaded ahead of the weights
            xst0 = []
            for kt in range(KT):
                t = xsp.tile([128, 512], f32r, tag=f"xs{kt}", bufs=2,
                             name=f"xa{kt}_0")
                nc.sync.dma_start(t[:], xT_d[kt * 128:(kt + 1) * 128, 0:512])
                xst0.append(t)
            # qk weight chunks, resident through A1
            wqk = []
            for kt in range(KT):
                w = wgtp.tile([128, 2 * C], f32r, tag=f"w{kt}", name=f"wqk{kt}")
                nc.sync.dma_start(w[:], wqkT_d[kt * 128:(kt + 1) * 128, :])
                wqk.append(w)

            # Gram accumulators: stA = heads 0,1 / stB = heads 2,3
            stA = ps2.tile([128, 1024], f32, tag="pE", name="stA")
            stB = ps2.tile([128, 1024], f32, tag="pF", name="stB")

            def st_slice(h, m):
                t = stA if h < 2 else stB
                off = (h % 2) * 512 + m * 256
                return t[:, off:off + 256]

            accq = wrk.tile([128, C], f32, tag="accq", name="accq")
            acck = wrk.tile([128, C], f32, tag="acck", name="acck")

            # ---------------- phase A1: q,k + Grams + sumsq ----------------
            for n5 in range(NT5):
                if n5 == 0:
                    xst = xst0
                else:
                    xst = []
                    for kt in range(KT):
                        t = xsp.tile([128, 512], f32r, tag=f"xs{kt}", bufs=2,
                                     name=f"xa{kt}_{n5}")
                        nc.sync.dma_start(
                            t[:], xT_d[kt * 128:(kt + 1) * 128,
                                       n5 * 512:(n5 + 1) * 512])
                        xst.append(t)
                for s in range(4):
                    tidx = n5 * 4 + s
                    qp0 = ps1.tile([128, 512], f32, tag="pA", name="qp0")
                    qp1 = ps1.tile([128, 512], f32, tag="pB", name="qp1")
                    kp0 = ps1.tile([128, 512], f32, tag="pC", name="kp0")
                    kp1 = ps1.tile([128, 512], f32, tag="pD", name="kp1")
                    for kt in range(KT):
                        lhs = xst[kt][:, s * 128:(s + 1) * 128]
                        fl, ll = (kt == 0), (kt == KT - 1)
                        nc.tensor.matmul(qp0[:], lhs, wqk[kt][:, 0:512],
                                         start=fl, stop=ll)
                        nc.tensor.matmul(qp1[:], lhs, wqk[kt][:, 512:1024],
                                         start=fl, stop=ll)
                        nc.tensor.matmul(kp0[:], lhs, wqk[kt][:, 1024:1536],
                                         start=fl, stop=ll)
                        nc.tensor.matmul(kp1[:], lhs, wqk[kt][:, 1536:2048],
                                         start=fl, stop=ll)
                    qcol = wrk.tile([128, C], f32r, tag="qcol", name="qcol")
                    kcol = wrk.tile([128, C], f32r, tag="kcol", name="kcol")
                    nc.vector.tensor_copy(qcol[:, 0:512], qp0[:])
                    nc.vector.tensor_copy(qcol[:, 512:1024], qp1[:])
                    nc.vector.tensor_copy(kcol[:, 0:512], kp0[:])
                    nc.vector.tensor_copy(kcol[:, 512:1024], kp1[:])
                    sq = wrk.tile([128, C], f32, tag="sq", name="sq")
                    sk = wrk.tile([128, C], f32, tag="sk", name="sk")
                    # square from the SBUF copies so the psum banks free
                    # after a single reader (keeps PE accumulation rolling)
                    nc.scalar.square(sq[:], qcol[:].bitcast(f32))
                    nc.scalar.square(sk[:], kcol[:].bitcast(f32))
                    if tidx == 0:
                        nc.gpsimd.tensor_copy(accq[:], sq[:])
                        nc.gpsimd.tensor_copy(acck[:], sk[:])
                    else:
                        nc.gpsimd.tensor_add(accq[:], accq[:], sq[:])
                        nc.gpsimd.tensor_add(acck[:], acck[:], sk[:])
                    for h in range(H):
                        for m in range(2):
                            nc.tensor.matmul(
                                st_slice(h, m),
                                kcol[:, h * 256 + m * 128: h * 256 + (m + 1) * 128],
                                qcol[:, h * 256:(h + 1) * 256],
                                start=(tidx == 0), stop=(tidx == 15),
                                skip_group_check=True)

            # sumsq rows: [1, 512] ones-matmuls into the freed qk psum slots
            ss_ps = []
            for i, (src, lo) in enumerate([(accq, 0), (accq, 512),
                                           (acck, 0), (acck, 512)]):
                sp = ps1.tile([1, 512], f32, tag=P1[i], name=f"ss{i}")
                nc.tensor.matmul(sp[:], ones_sb[:], src[:, lo:lo + 512],
                                 start=True, stop=True)
                ss_ps.append(sp)

            # SBUF bounces for the collective input (DMA cannot read PSUM);
            # all land in slots whose previous tenants just died.
            stA_sb = wrk.tile([128, 1024], f32, tag="qcol", name="stA_sb")
            stB_sb = wrk.tile([128, 1024], f32, tag="kcol", name="stB_sb")
            nc.vector.tensor_copy(stA_sb[:], stA[:])
            nc.vector.tensor_copy(stB_sb[:], stB[:])
            ss_sb = []
            for i, tg in enumerate(["sq", "sk", "accq", "acck"]):
                sb = wrk.tile([1, 512], f32, tag=tg, name=f"ssb{i}")
                nc.vector.tensor_copy(sb[:], ss_ps[i][:])
                ss_sb.append(sb)

            # ---------------- AllReduce over batch-pairs ----------------
            CCN = 128 * 2048 + 2 * C
            cc_in = dramp.tile([CCN], f32, name="cc_in")
            cc_out = dramp.tile([CCN], f32, name="cc_out")
            nc.sync.dma_start(
                cc_in[0:131072].rearrange("(p f) -> p f", p=128), stA_sb[:])
            nc.sync.dma_start(
                cc_in[131072:262144].rearrange("(p f) -> p f", p=128), stB_sb[:])
            for i in range(4):
                nc.sync.dma_start(
                    cc_in[262144 + i * 512: 262144 + (i + 1) * 512]
                    .rearrange("(a f) -> a f", a=1), ss_sb[i][:])
            nc.gpsimd.collective_compute(
                "AllReduce", ADD,
                replica_groups=[[0, 1], [2, 3], [4, 5], [6, 7]],
                ins=[cc_in.opt()], outs=[cc_out.opt()])
            strA = wrk.tile([128, 1024], f32, tag="qcol", name="strA")
            strB = wrk.tile([128, 1024], f32, tag="kcol", name="strB")
            nc.sync.dma_start(
                strA[:], cc_out[0:131072].rearrange("(p f) -> p f", p=128))
            nc.sync.dma_start(
                strB[:], cc_out[131072:262144].rearrange("(p f) -> p f", p=128))
            ssred = constp.tile([128, 16], f32, name="ssred")
            nc.sync.dma_start(
                ssred[:],
                cc_out[262144:262144 + 2048].rearrange("(j p) -> p j", p=128))

            def str_slice(h, m):
                t = strA if h < 2 else strB
                off = (h % 2) * 512 + m * 256
                return t[:, off:off + 256]

            # ---------------- phase A2: v (overlaps the collective) -------
            wv = []
            for kt in range(KT):
                w = wgtp.tile([128, C], f32r, tag=f"w{kt}", name=f"wv{kt}")
                nc.sync.dma_start(w[:], wvT_d[kt * 128:(kt + 1) * 128, :])
                wv.append(w)
            v_sb = [vop.tile([128, NL], f32r, tag=f"vo{cv}", name=f"v{cv}")
                    for cv in range(8)]
            pcnt = 0
            for pb in range(2):
                xst = []
                for kt in range(KT):
                    ta = xsp.tile([128, 512], f32r, tag=f"xs{kt}", bufs=2,
                                  name=f"xva{kt}_{pb}")
                    tb = xsp.tile([128, 512], f32r, tag=f"xs{kt}", bufs=2,
                                  name=f"xvb{kt}_{pb}")
                    nc.sync.dma_start(
                        ta[:], xT_d[kt * 128:(kt + 1) * 128,
                                    pb * 1024: pb * 1024 + 512])
                    nc.sync.dma_start(
                        tb[:], xT_d[kt * 128:(kt + 1) * 128,
                                    pb * 1024 + 512: pb * 1024 + 1024])
                    xst.append((ta, tb))
                for cv in range(8):
                    va = ps1.tile([128, 512], f32, tag=P1[pcnt % 4], name="vpa")
                    pcnt += 1
                    vb = ps1.tile([128, 512], f32, tag=P1[pcnt % 4], name="vpb")
                    pcnt += 1
                    for kt in range(KT):
                        fl, ll = (kt == 0), (kt == KT - 1)
                        nc.tensor.matmul(va[:],
                                         wv[kt][:, cv * 128:(cv + 1) * 128],
                                         xst[kt][0][:], start=fl, stop=ll)
                        nc.tensor.matmul(vb[:],
                                         wv[kt][:, cv * 128:(cv + 1) * 128],
                                         xst[kt][1][:], start=fl, stop=ll)
                    nc.vector.tensor_copy(
                        v_sb[cv][:, pb * 1024: pb * 1024 + 512], va[:])
                    nc.vector.tensor_copy(
                        v_sb[cv][:, pb * 1024 + 512: pb * 1024 + 1024], vb[:])

            # ---------------- phase B: normalize + softmax + attn@v -------
            # rq = temp/max(sqrt(ssq),eps), rk = 1/max(sqrt(ssk),eps), as
            # per-partition columns [128, 16]: cols 0-7 = rq, 8-15 = rk.
            rqk = constp.tile([128, 16], f32, name="rqk")
            nc.scalar.sqrt(rqk[:], ssred[:])
            nc.vector.tensor_scalar_max(rqk[:], rqk[:], EPS)
            nc.vector.reciprocal(rqk[:], rqk[:])
            nc.vector.tensor_mul(rqk[:, 0:8], rqk[:, 0:8], tmpv_sb[:])

            outT = []
            for h in range(H):
                # Gram^T rows d scaled by rk[d]
                sth = xsp.tile([128, 512], f32, tag="xs4", bufs=2, name="sth")
                for m in range(2):
                    nc.vector.tensor_scalar_mul(
                        sth[:, m * 256:(m + 1) * 256], str_slice(h, m),
                        rqk[:, 8 + 2 * h + m: 9 + 2 * h + m])
                # transpose to S[c, d]
                spm = ps1.tile([128, 512], f32, tag="pA", name="spm")
                for mc in range(2):
                    for md in range(2):
                        nc.tensor.transpose(
                            spm[:, mc * 256 + md * 128: mc * 256 + (md + 1) * 128],
                            sth[:, md * 256 + mc * 128: md * 256 + (mc + 1) * 128],
                            ident[:])
                sft = xsp.tile([128, 512], f32, tag="xs5", bufs=2, name="sft")
                for mc in range(2):
                    nc.vector.tensor_scalar_mul(
                        sft[:, mc * 256:(mc + 1) * 256],
                        spm[:, mc * 256:(mc + 1) * 256],
                        rqk[:, 2 * h + mc: 1 + 2 * h + mc])
                # softmax over d (free axis)
                negmax = wrk.tile([128, 2], f32, tag="negmax", name="negmax")
                rowsum = wrk.tile([128, 2], f32, tag="rowsum", name="rowsum")
                recip = wrk.tile([128, 2], f32, tag="recip", name="recip")
                esb = xsp.tile([128, 512], f32, tag="xs6", bufs=2, name="esb")
                for mc in range(2):
                    nc.vector.reduce_max(negmax[:, mc:mc + 1],
                                         sft[:, mc * 256:(mc + 1) * 256],
                                         axis=AX, negate=True)
                    nc.scalar.activation(esb[:, mc * 256:(mc + 1) * 256],
                                         sft[:, mc * 256:(mc + 1) * 256],
                                         Exp, bias=negmax[:, mc:mc + 1],
                                         accum_out=rowsum[:, mc:mc + 1])
                nc.vector.reciprocal(recip[:], rowsum[:])
                # attn^T (columns d on partitions)
                atp = ps1.tile([128, 512], f32, tag="pB", name="atp")
                for md in range(2):
                    for mc in range(2):
                        nc.tensor.transpose(
                            atp[:, md * 256 + mc * 128: md * 256 + (mc + 1) * 128],
                            esb[:, mc * 256 + md * 128: mc * 256 + (md + 1) * 128],
                            ident[:])
                atn = xsp.tile([128, 512], f32r, tag="xs7", bufs=2, name="atn")
                nc.vector.tensor_copy(atn[:], atp[:])
                # out^T[c,:] = sum_d attn^T[d,c] v[d,:], row-scaled by 1/rowsum
                op2s = {}
                for mc in range(2):
                    for nfh in range(2):
                        op2 = ps2.tile([128, 1024], f32,
                                       tag=("pE" if nfh == 0 else "pF"),
                                       name="op2")
                        op2s[(mc, nfh)] = op2
                        for md in range(2):
                            for n2 in range(2):
                                nc.tensor.matmul(
                                    op2[:, n2 * 512:(n2 + 1) * 512],
                                    atn[:, md * 256 + mc * 128: md * 256 + (mc + 1) * 128],
                                    v_sb[2 * h + md][:, nfh * 1024 + n2 * 512:
                                                     nfh * 1024 + (n2 + 1) * 512],
                                    start=(md == 0), stop=(md == 1))
                for mc in range(2):
                    i = 2 * h + mc
                    ot = vop.tile([128, NL], f32r, tag=f"vo{(i + 8) % 9}",
                                  name=f"ot{i}")
                    outT.append(ot)
                    for nfh in range(2):
                        nc.vector.tensor_scalar_mul(
                            ot[:, nfh * 1024:(nfh + 1) * 1024],
                            op2s[(mc, nfh)][:], recip[:, mc:mc + 1])

            # ---------------- phase C: projection + bias + residual -------
            wp = []
            for kt in range(KT):
                w = wgtp.tile([128, C], f32r, tag=f"w{kt}", name=f"wp{kt}")
                nc.sync.dma_start(w[:], wpT_d[kt * 128:(kt + 1) * 128, :])
                wp.append(w)
            for j in range(KT):
                pq = []
                for q in range(4):
                    p = ps1.tile([128, 512], f32, tag=P1[q], name=f"pp{q}")
                    pq.append(p)
                for kt in range(KT):
                    # proj input channel chunk kt = (jj=kt//2, d-half=kt%2);
                    # column block q is head q; tokens subsampled jj::4
                    for q in range(4):
                        nc.tensor.matmul(
                            pq[q][:],
                            wp[kt][:, j * 128:(j + 1) * 128],
                            outT[2 * q + kt % 2][:, (kt // 2)::4],
                            start=(kt == 0), stop=(kt == KT - 1))
                for q in range(4):
                    xr = xsp.tile([128, 512], f32r, tag=f"xs{4 + q}", bufs=2,
                                  name=f"xr{j}_{q}")
                    nc.sync.dma_start(
                        xr[:], xrT_d[j * 128:(j + 1) * 128,
                                     q * 512:(q + 1) * 512])
                    yq = xsp.tile([128, 512], f32, tag=f"xs{q}", bufs=2,
                                  name=f"yq{j}_{q}")
                    nc.scalar.activation(yq[:], pq[q][:], Ident,
                                         bias=bias_sb[:, j:j + 1])
                    nc.vector.tensor_add(yq[:], yq[:], xr[:].bitcast(f32))
                    nc.sync.dma_start(
                        yT_d[j * 128:(j + 1) * 128, q * 512:(q + 1) * 512],
                        yq[:])

    nc.compile()
    return nc


def _get_nc():
    if "nc" not in _CACHE:
        _CACHE["nc"] = _build()
    return _CACHE["nc"]


def _make_in_maps(x, Wqkv, Wproj, bproj, temperature):
    x = np.ascontiguousarray(np.asarray(x, dtype=np.float32))
    Wqkv = np.asarray(Wqkv, dtype=np.float32)
    Wproj = np.asarray(Wproj, dtype=np.float32)
    bproj = np.asarray(bproj, dtype=np.float32).reshape(C)
    temp = np.asarray(temperature, dtype=np.float32).reshape(H)

    WqkvT = np.ascontiguousarray(Wqkv.T)          # [C, 3C]
    wqkT = np.ascontiguousarray(WqkvT[:, :2 * C])
    wvT = np.ascontiguousarray(WqkvT[:, 2 * C:])
    wpT = np.ascontiguousarray(Wproj.T)
    bias2d = np.ascontiguousarray(bproj.reshape(KT, 128).T)
    tmpv2d = np.ascontiguousarray(np.repeat(temp, HD).reshape(KT, 128).T)

    in_maps = []
    for core in range(NCORES):
        b, half = core // 2, core % 2
        xT = np.ascontiguousarray(x[b, half * NL:(half + 1) * NL, :].T)
        rows = _out_rows(half)
        xrT = np.ascontiguousarray(x[b, rows, :].T)
        in_maps.append(dict(xT=xT, xrT=xrT, wqkT=wqkT, wvT=wvT, wpT=wpT,
                            bias=bias2d, tmpv=tmpv2d))
    return in_maps


def _out_rows(half):
    # torch transpose+reshape scramble: this core's y rows
    return np.concatenate(
        [h * 1024 + half * 512 + np.arange(512) for h in range(H)])


def _run(in_maps, trace=False, **kw):
    from concourse.bass_utils import run_bass_kernel_spmd

    nc = _get_nc()
    return run_bass_kernel_spmd(nc, in_maps, core_ids=list(range(NCORES)),
                                trace=trace, **kw)


def kernel(x, Wqkv, Wproj, bproj, temperature):
    res = _run(_make_in_maps(x, Wqkv, Wproj, bproj, temperature))
    y = np.empty((B, N, C), dtype=np.float32)
    for core in range(NCORES):
        b, half = core // 2, core % 2
        y[b, _out_rows(half), :] = res.results[core]["yT"].T
    return y



# revision 8
# speedup vs baseline: 1.8646x; 1.8646x over previous
"""Efficient Channel Attention kernel for 8 Trainium2 NeuronCores.

Problem (B=4, N=4096, C=1024, H=4, HD=256):
    qkv = x @ Wqkv.T                 -> q,k,v per head
    q,k l2-normalized over N; scores = (q*temp) @ k.T   [HD, HD] per (b,h)
    attn = softmax(scores, -1); out = attn @ v; y = out @ Wproj.T + bproj + x

Sharding: core = (batch b, token-half). All channel contractions are local;
the only cross-core coupling is the token(N)-contracted quantities: the raw
Grams k^T q and the q/k squared norms, AllReduce'd (bf16, ~0.5MB) within the
core pair sharing a batch.

All large GEMMs run in fp8(e4m3) with MatmulPerfMode.DoubleRow: operands are
laid out [K=128, 2, free] (two 128-row contraction groups per instruction,
0.5 cycles/row = 2x bf16/f32r PE rate). Weights are pre-scaled by WS=64 on
the host so W entries sit in fp8's normal range; the resulting power-of-two
output scales are folded into the norm reciprocals (scores) and the final
projection bias-activation (scale=1/4096).

PSUM discipline: exactly 4 tags x [128,1024] x 1 buf = 8 banks, manually
alternated (pA/pB vs pC/pD) for double buffering in each phase.
"""

import numpy as np
import ml_dtypes

B, N, C, H = 4, 4096, 1024, 4
HD = C // H          # 256
NCORES = 8
NL = N // 2          # 2048 tokens per core
WS = 32.0            # host-side weight prescale (q/k/v reach ~7.7 abs;
                     # 32x keeps everything well under fp8 e4m3's 448 max)
F8 = ml_dtypes.float8_e4m3
BF16 = ml_dtypes.bfloat16

_CACHE = {}


def _build():
    import concourse.mybir as mybir
    import concourse.tile as tile
    from concourse import bacc
    from concourse.masks import make_identity

    f32 = mybir.dt.float32
    bf = mybir.dt.bfloat16
    f8 = mybir.dt.float8e4
    DR = mybir.MatmulPerfMode.DoubleRow
    AX = mybir.AxisListType.X
    ADD = mybir.AluOpType.add
    Exp = mybir.ActivationFunctionType.Exp
    Ident = mybir.ActivationFunctionType.Identity
    Square = mybir.ActivationFunctionType.Square
    Sqrt = mybir.ActivationFunctionType.Sqrt

    nc = bacc.Bacc("TRN2", target_bir_lowering=False, debug=False,
                   num_devices=NCORES)

    x8_d = nc.dram_tensor("x8", [512, 2, NL], f8, kind="ExternalInput").ap()
    wqk8_d = nc.dram_tensor("wqk8", [512, 2, 2048], f8, kind="ExternalInput").ap()
    wv8_d = nc.dram_tensor("wv8", [512, 2, 1024], f8, kind="ExternalInput").ap()
    wp8_d = nc.dram_tensor("wp8", [512, 2, 1024], f8, kind="ExternalInput").ap()
    xr_d = nc.dram_tensor("xr", [C, NL], bf, kind="ExternalInput").ap()
    bias_d = nc.dram_tensor("bias", [128, 8], f32, kind="ExternalInput").ap()
    tmpv_d = nc.dram_tensor("tmpv", [128, 8], f32, kind="ExternalInput").ap()
    yT_d = nc.dram_tensor("yT", [C, NL], f32, kind="ExternalOutput").ap()

    with tile.TileContext(nc) as tc:
        with (
            tc.tile_pool(name="const", bufs=1) as constp,
            tc.tile_pool(name="wgt", bufs=1) as wgtp,
            tc.tile_pool(name="xs", bufs=1) as xsp,
            tc.tile_pool(name="qk", bufs=1) as qkp,
            tc.tile_pool(name="vo", bufs=1) as vop,
            tc.tile_pool(name="wrk", bufs=1) as wrk,
            tc.tile_pool(name="ps1", bufs=1, space="PSUM") as ps1,
            tc.tile_pool(name="dram", bufs=1, space="DRAM") as dramp,
        ):
            # ---------------- constants ----------------
            ident = constp.tile([128, 128], f32, name="ident")
            make_identity(nc, ident[:])
            identb = constp.tile([128, 128], bf, name="identb")
            nc.gpsimd.tensor_copy(identb[:], ident[:])
            ident8 = constp.tile([128, 128], f8, name="ident8")
            nc.gpsimd.tensor_copy(ident8[:], ident[:])
            bias_sb = constp.tile([128, 8], f32, name="bias_sb")
            nc.sync.dma_start(bias_sb[:], bias_d[:])
            tmpv_sb = constp.tile([128, 8], f32, name="tmpv_sb")
            nc.sync.dma_start(tmpv_sb[:], tmpv_d[:])
            ones8 = constp.tile([128, 2, 128], f8, name="ones8")
            nc.vector.memset(ones8[:], 1.0)

            # ---------------- bulk input DMA (all SBUF-resident) ---------
            x8 = []
            for t in range(4):
                xt = xsp.tile([128, 2, NL], f8, tag=f"x{t}", name=f"x8_{t}")
                nc.sync.dma_start(xt[:], x8_d[t * 128:(t + 1) * 128])
                x8.append(xt)
            wqk8 = []
            for t in range(4):
                wt = wgtp.tile([128, 2, 2048], f8, tag=f"wqk{t}", name=f"wqk8_{t}")
                nc.sync.dma_start(wt[:], wqk8_d[t * 128:(t + 1) * 128])
                wqk8.append(wt)
            wv8 = []
            for t in range(4):
                wt = wgtp.tile([128, 2, 1024], f8, tag=f"wv{t}", name=f"wv8_{t}")
                nc.sync.dma_start(wt[:], wv8_d[t * 128:(t + 1) * 128])
                wv8.append(wt)
            wp8 = []
            for t in range(4):
                wt = wgtp.tile([128, 2, 1024], f8, tag=f"wp{t}", name=f"wp8_{t}")
                nc.sync.dma_start(wt[:], wp8_d[t * 128:(t + 1) * 128])
                wp8.append(wt)

            P = ["pA", "pB", "pC", "pD"]  # 4 x [128,1024] = 8 banks

            # ---------------- A1a: q,k -> fp8 (+ squares) ----------------
            # qk8[u][p, sl, 0:1024]=q / [1024:2048]=k for token tile 2u+sl;
            # sqk8 = (q/64)^2 etc. (true, unscaled squares; fp8-safe range)
            qk8, sqk8 = [], []
            for u in range(8):
                qt = qkp.tile([128, 2, 2048], f8, tag=f"qk{u}", name=f"qk8_{u}")
                st = qkp.tile([128, 2, 2048], f8, tag=f"sqk{u}", name=f"sqk8_{u}")
                qk8.append(qt)
                sqk8.append(st)
            for u in range(8):
                for sl in range(2):
                    s = 2 * u + sl
                    q_ps = ps1.tile([128, 1024], f32, tag=P[2 * sl], name="q_ps")
                    k_ps = ps1.tile([128, 1024], f32, tag=P[2 * sl + 1], name="k_ps")
                    for c in range(2):
                        for t in range(4):
                            nc.tensor.matmul(
                                q_ps[:, c * 512:(c + 1) * 512],
                                x8[t][:, :, s * 128:(s + 1) * 128],
                                wqk8[t][:, :, c * 512:(c + 1) * 512],
                                start=(t == 0), stop=(t == 3), perf_mode=DR)
                    for c in range(2):
                        for t in range(4):
                            nc.tensor.matmul(
                                k_ps[:, c * 512:(c + 1) * 512],
                                x8[t][:, :, s * 128:(s + 1) * 128],
                                wqk8[t][:, :, 1024 + c * 512:1024 + (c + 1) * 512],
                                start=(t == 0), stop=(t == 3), perf_mode=DR)
                    nc.vector.tensor_copy(qk8[u][:, sl, 0:1024], q_ps[:])
                    nc.vector.tensor_copy(qk8[u][:, sl, 1024:2048], k_ps[:])
                    nc.scalar.activation(sqk8[u][:, sl, 0:1024], q_ps[:],
                                         Square, scale=1.0 / WS)
                    nc.scalar.activation(sqk8[u][:, sl, 1024:2048], k_ps[:],
                                         Square, scale=1.0 / WS)

            # ---------------- A1b: Grams + sumsq --------------------------
            # stA = heads 0,1 / stB = heads 2,3; Gram'[kc,qc] = sum k'q'
            stA = ps1.tile([128, 1024], f32, tag="pA", name="stA")
            stB = ps1.tile([128, 1024], f32, tag="pB", name="stB")
            # ones stationary is M=128 wide (narrow DR ldweights fails the
            # ISA check); every psum row holds the same channel sums
            ssq_q = ps1.tile([128, 1024], f32, tag="pC", name="ssq_q")
            ssq_k = ps1.tile([128, 1024], f32, tag="pD", name="ssq_k")

            def st_slice(h, m):
                t = stA if h < 2 else stB
                off = (h % 2) * 512 + m * 256
                return t[:, off:off + 256]

            for h in range(H):
                for m in range(2):
                    for u in range(8):
                        nc.tensor.matmul(
                            st_slice(h, m),
                            qk8[u][:, :, 1024 + h * 256 + m * 128:
                                   1024 + h * 256 + (m + 1) * 128],
                            qk8[u][:, :, h * 256:(h + 1) * 256],
                            start=(u == 0), stop=(u == 7), perf_mode=DR)
            for c in range(2):
                for u in range(8):
                    nc.tensor.matmul(ssq_q[:, c * 512:(c + 1) * 512], ones8[:],
                                     sqk8[u][:, :, c * 512:(c + 1) * 512],
                                     start=(u == 0), stop=(u == 7), perf_mode=DR)
                    nc.tensor.matmul(ssq_k[:, c * 512:(c + 1) * 512], ones8[:],
                                     sqk8[u][:, :, 1024 + c * 512:1024 + (c + 1) * 512],
                                     start=(u == 0), stop=(u == 7), perf_mode=DR)

            # bf16 SBUF bounces for the collective (DMA cannot read PSUM)
            stA_sb = wrk.tile([128, 1024], bf, tag="stA_sb", name="stA_sb")
            stB_sb = wrk.tile([128, 1024], bf, tag="stB_sb", name="stB_sb")
            ssq_sb = wrk.tile([1, 2048], bf, tag="ssq_sb", name="ssq_sb")
            nc.vector.tensor_copy(stA_sb[:], stA[:])
            nc.vector.tensor_copy(stB_sb[:], stB[:])
            nc.vector.tensor_copy(ssq_sb[:, 0:1024], ssq_q[0:1, :])
            nc.vector.tensor_copy(ssq_sb[:, 1024:2048], ssq_k[0:1, :])

            # ---------------- AllReduce over batch-pairs (bf16) -----------
            CCN = 128 * 2048 + 2048
            cc_in = dramp.tile([CCN], bf, name="cc_in")
            cc_out = dramp.tile([CCN], bf, name="cc_out")
            nc.sync.dma_start(
                cc_in[0:131072].rearrange("(p f) -> p f", p=128), stA_sb[:])
            nc.sync.dma_start(
                cc_in[131072:262144].rearrange("(p f) -> p f", p=128), stB_sb[:])
            nc.sync.dma_start(
                cc_in[262144:264192].rearrange("(a f) -> a f", a=1), ssq_sb[:])
            nc.gpsimd.collective_compute(
                "AllReduce", ADD,
                replica_groups=[[0, 1], [2, 3], [4, 5], [6, 7]],
                ins=[cc_in.opt()], outs=[cc_out.opt()])
            strA = wrk.tile([128, 1024], bf, tag="stA_sb", name="strA")
            strB = wrk.tile([128, 1024], bf, tag="stB_sb", name="strB")
            nc.sync.dma_start(
                strA[:], cc_out[0:131072].rearrange("(p f) -> p f", p=128))
            nc.sync.dma_start(
                strB[:], cc_out[131072:262144].rearrange("(p f) -> p f", p=128))
            ssred = constp.tile([128, 16], bf, name="ssred")
            nc.sync.dma_start(
                ssred[:],
                cc_out[262144:264192].rearrange("(j p) -> p j", p=128))

            def str_slice(h, m):
                t = strA if h < 2 else strB
                off = (h % 2) * 512 + m * 256
                return t[:, off:off + 256]

            # ---------------- A2: v (overlaps the collective) -------------
            # v8[h][p, i, tok] = v'[h*256 + i*128 + p, tok]  (fp8, x64)
            v8 = []
            for h in range(H):
                vt = vop.tile([128, 2, NL], f8, tag=f"v{h}", name=f"v8_{h}")
                v8.append(vt)
            for cv in range(8):
                h, i = cv // 2, cv % 2
                vp0 = ps1.tile([128, 1024], f32, tag=P[2 * (cv % 2)], name="vp0")
                vp1 = ps1.tile([128, 1024], f32, tag=P[2 * (cv % 2) + 1], name="vp1")
                for half, vp in ((0, vp0), (1, vp1)):
                    for c in range(2):
                        for t in range(4):
                            nc.tensor.matmul(
                                vp[:, c * 512:(c + 1) * 512],
                                wv8[t][:, :, cv * 128:(cv + 1) * 128],
                                x8[t][:, :, half * 1024 + c * 512:
                                      half * 1024 + (c + 1) * 512],
                                start=(t == 0), stop=(t == 3), perf_mode=DR)
                nc.vector.tensor_copy(v8[h][:, i, 0:1024], vp0[:])
                nc.vector.tensor_copy(v8[h][:, i, 1024:2048], vp1[:])

            # ---------------- phase B: softmax + attn@v -------------------
            # rqk cols 0-7: rq = temp/max(64*||q||,eps); 8-15: rk = 1/(64||k||)
            rqk = constp.tile([128, 16], f32, name="rqk")
            nc.scalar.activation(rqk[:], ssred[:], Sqrt, scale=WS * WS)
            nc.vector.tensor_scalar_max(rqk[:], rqk[:], 1e-9)
            nc.vector.reciprocal(rqk[:], rqk[:])
            nc.vector.tensor_mul(rqk[:, 0:8], rqk[:, 0:8], tmpv_sb[:])

            # out8 reuses x8's SBUF slots (x8 has no readers after A2)
            out8 = []
            for h in range(H):
                ot = xsp.tile([128, 2, NL], f8, tag=f"x{h}", name=f"out8_{h}")
                out8.append(ot)
            for h in range(H):
                # rows d scaled by rk[d] -> bf16
                sth = wrk.tile([128, 512], bf, tag="sth", bufs=2, name="sth")
                for m in range(2):
                    nc.vector.tensor_scalar_mul(
                        sth[:, m * 256:(m + 1) * 256], str_slice(h, m),
                        rqk[:, 8 + 2 * h + m: 9 + 2 * h + m])
                # transpose to S[c, d] (bf16)
                spm = ps1.tile([128, 512], bf, tag=P[2 * (h % 2)], name="spm")
                for mc in range(2):
                    for md in range(2):
                        nc.tensor.transpose(
                            spm[:, mc * 256 + md * 128: mc * 256 + (md + 1) * 128],
                            sth[:, md * 256 + mc * 128: md * 256 + (mc + 1) * 128],
                            identb[:])
                sft = wrk.tile([128, 512], f32, tag="sft", bufs=2, name="sft")
                for mc in range(2):
                    nc.vector.tensor_scalar_mul(
                        sft[:, mc * 256:(mc + 1) * 256],
                        spm[:, mc * 256:(mc + 1) * 256],
                        rqk[:, 2 * h + mc: 1 + 2 * h + mc])
                # softmax over d (free axis); exp output straight to fp8
                negmax = wrk.tile([128, 2], f32, tag="negmax", bufs=2, name="negmax")
                rowsum = wrk.tile([128, 2], f32, tag="rowsum", bufs=2, name="rowsum")
                recip = wrk.tile([128, 2], f32, tag="recip", bufs=2, name="recip")
                esb = wrk.tile([128, 512], bf, tag="esb", bufs=2, name="esb")
                for mc in range(2):
                    nc.vector.reduce_max(negmax[:, mc:mc + 1],
                                         sft[:, mc * 256:(mc + 1) * 256],
                                         axis=AX, negate=True)
                    nc.scalar.activation(esb[:, mc * 256:(mc + 1) * 256],
                                         sft[:, mc * 256:(mc + 1) * 256],
                                         Exp, bias=negmax[:, mc:mc + 1],
                                         accum_out=rowsum[:, mc:mc + 1])
                nc.vector.reciprocal(recip[:], rowsum[:])
                # attn^T [d, (md, c)] via bf16 PE transposes (fp8 transpose
                # needs stride-2 psum writes per the walrus verifier)
                atp = ps1.tile([128, 512], bf, tag=P[2 * (h % 2) + 1], name="atp")
                for md in range(2):
                    for mc in range(2):
                        nc.tensor.transpose(
                            atp[:, md * 256 + mc * 128: md * 256 + (mc + 1) * 128],
                            esb[:, mc * 256 + md * 128: mc * 256 + (md + 1) * 128],
                            identb[:])
                atn8 = wrk.tile([128, 2, 256], f8, tag="atn8", bufs=2, name="atn8")
                nc.vector.tensor_copy(atn8[:, 0, :], atp[:, 0:256])
                nc.vector.tensor_copy(atn8[:, 1, :], atp[:, 256:512])
                # out' = attn^T(unnorm) @ v', row-scaled by 1/rowsum later
                for nfh in range(2):
                    opc = ps1.tile([128, 1024], f32, tag=P[2 * (h % 2)], name="opc")
                    opd = ps1.tile([128, 1024], f32, tag=P[2 * (h % 2) + 1], name="opd")
                    for mc, op in ((0, opc), (1, opd)):
                        for nf2 in range(2):
                            nc.tensor.matmul(
                                op[:, nf2 * 512:(nf2 + 1) * 512],
                                atn8[:, :, mc * 128:(mc + 1) * 128],
                                v8[h][:, :, nfh * 1024 + nf2 * 512:
                                      nfh * 1024 + (nf2 + 1) * 512],
                                start=True, stop=True, perf_mode=DR)
                    for mc, op in ((0, opc), (1, opd)):
                        nc.vector.tensor_scalar_mul(
                            out8[h][:, mc, nfh * 1024:(nfh + 1) * 1024],
                            op[:], recip[:, mc:mc + 1])

            # ---------------- phase C: projection + bias + residual -------
            for j in range(8):
                xrj = xsp.tile([128, NL], bf, tag=f"xr{j % 2}", bufs=2,
                               name=f"xr_{j}")
                nc.sync.dma_start(xrj[:], xr_d[j * 128:(j + 1) * 128, :])
                pq01 = ps1.tile([128, 1024], f32, tag=P[2 * (j % 2)], name="pq01")
                pq23 = ps1.tile([128, 1024], f32, tag=P[2 * (j % 2) + 1], name="pq23")
                for q in range(4):
                    pt = pq01 if q < 2 else pq23
                    off = (q % 2) * 512
                    for u in range(4):
                        nc.tensor.matmul(
                            pt[:, off:off + 512],
                            wp8[u][:, :, j * 128:(j + 1) * 128],
                            out8[q][:, :, u::4],
                            start=(u == 0), stop=(u == 3), perf_mode=DR)
                yj = wrk.tile([128, NL], f32, tag=f"yj{j % 2}", bufs=2,
                              name=f"yj_{j}")
                nc.scalar.activation(yj[:, 0:1024], pq01[:], Ident,
                                     bias=bias_sb[:, j:j + 1],
                                     scale=1.0 / (WS * WS))
                nc.scalar.activation(yj[:, 1024:2048], pq23[:], Ident,
                                     bias=bias_sb[:, j:j + 1],
                                     scale=1.0 / (WS * WS))
                nc.vector.tensor_add(yj[:], yj[:], xrj[:])
                nc.sync.dma_start(yT_d[j * 128:(j + 1) * 128, :], yj[:])

    nc.compile()
    return nc


def _get_nc():
    if "nc" not in _CACHE:
        _CACHE["nc"] = _build()
    return _CACHE["nc"]


def _dr_pack(a):
    """[C, F] channel-major -> DoubleRow [512, 2, F]: row t*128+p carries
    channels (256t+p, 256t+128+p) group-major."""
    Cc, F = a.shape
    return np.ascontiguousarray(
        a.reshape(Cc // 256, 2, 128, F).transpose(0, 2, 1, 3).reshape(
            Cc // 2, 2, F))


def _out_rows(half):
    # torch transpose+reshape scramble: this core's y rows
    return np.concatenate(
        [h * 1024 + half * 512 + np.arange(512) for h in range(H)])


def _make_in_maps(x, Wqkv, Wproj, bproj, temperature):
    x = np.ascontiguousarray(np.asarray(x, dtype=np.float32))
    Wqkv = np.asarray(Wqkv, dtype=np.float32)
    Wproj = np.asarray(Wproj, dtype=np.float32)
    bproj = np.asarray(bproj, dtype=np.float32).reshape(C)
    temp = np.asarray(temperature, dtype=np.float32).reshape(H)

    WqkvT = np.ascontiguousarray(Wqkv.T) * WS     # [C, 3C], prescaled
    wqk8 = _dr_pack(WqkvT[:, :2 * C]).astype(F8)
    wv8 = _dr_pack(WqkvT[:, 2 * C:]).astype(F8)
    wp8 = _dr_pack(np.ascontiguousarray(Wproj.T) * WS).astype(F8)
    bias2d = np.ascontiguousarray(bproj.reshape(8, 128).T)
    tmpv2d = np.ascontiguousarray(np.repeat(temp, HD).reshape(8, 128).T)

    in_maps = []
    for core in range(NCORES):
        b, half = core // 2, core % 2
        xT = np.ascontiguousarray(x[b, half * NL:(half + 1) * NL, :].T)
        x8 = _dr_pack(xT).astype(F8)
        xr = np.ascontiguousarray(x[b, _out_rows(half), :].T).astype(BF16)
        in_maps.append(dict(x8=x8, xr=xr, wqk8=wqk8, wv8=wv8, wp8=wp8,
                            bias=bias2d, tmpv=tmpv2d))
    return in_maps


def _run(in_maps, trace=False, **kw):
    from concourse.bass_utils import run_bass_kernel_spmd

    nc = _get_nc()
    return run_bass_kernel_spmd(nc, in_maps, core_ids=list(range(NCORES)),
                                trace=trace, **kw)


def kernel(x, Wqkv, Wproj, bproj, temperature):
    res = _run(_make_in_maps(x, Wqkv, Wproj, bproj, temperature))
    y = np.empty((B, N, C), dtype=np.float32)
    for core in range(NCORES):
        b, half = core // 2, core % 2
        y[b, _out_rows(half), :] = res.results[core]["yT"].T
    return y


# revision 10
# speedup vs baseline: 1.9297x; 1.0349x over previous
"""Efficient Channel Attention kernel for 8 Trainium2 NeuronCores.

Problem (B=4, N=4096, C=1024, H=4, HD=256):
    qkv = x @ Wqkv.T                 -> q,k,v per head
    q,k l2-normalized over N; scores = (q*temp) @ k.T   [HD, HD] per (b,h)
    attn = softmax(scores, -1); out = attn @ v; y = out @ Wproj.T + bproj + x

Sharding: core = (batch b, token-half). All channel contractions are local;
the only cross-core coupling is the token(N)-contracted quantities: the raw
Grams k^T q and the q/k squared norms, AllReduce'd (bf16, ~0.5MB) within the
core pair sharing a batch.

All large GEMMs run in fp8(e4m3) with MatmulPerfMode.DoubleRow: operands are
laid out [K=128, 2, free] (two 128-row contraction groups per instruction,
0.5 cycles/row = 2x bf16/f32r PE rate). Weights are pre-scaled by WS=64 on
the host so W entries sit in fp8's normal range; the resulting power-of-two
output scales are folded into the norm reciprocals (scores) and the final
projection bias-activation (scale=1/4096).

PSUM discipline: exactly 4 tags x [128,1024] x 1 buf = 8 banks, manually
alternated (pA/pB vs pC/pD) for double buffering in each phase.
"""

import numpy as np
import ml_dtypes

B, N, C, H = 4, 4096, 1024, 4
HD = C // H          # 256
NCORES = 8
NL = N // 2          # 2048 tokens per core
WS = 32.0            # host-side weight prescale (q/k/v reach ~7.7 abs;
                     # 32x keeps everything well under fp8 e4m3's 448 max)
F8 = ml_dtypes.float8_e4m3
BF16 = ml_dtypes.bfloat16

_CACHE = {}


def _build():
    import concourse.mybir as mybir
    import concourse.tile as tile
    from concourse import bacc
    from concourse.masks import make_identity

    f32 = mybir.dt.float32
    bf = mybir.dt.bfloat16
    f8 = mybir.dt.float8e4
    DR = mybir.MatmulPerfMode.DoubleRow
    AX = mybir.AxisListType.X
    ADD = mybir.AluOpType.add
    Exp = mybir.ActivationFunctionType.Exp
    Ident = mybir.ActivationFunctionType.Identity
    Square = mybir.ActivationFunctionType.Square
    Sqrt = mybir.ActivationFunctionType.Sqrt

    nc = bacc.Bacc("TRN2", target_bir_lowering=False, debug=False,
                   num_devices=NCORES)

    x8_d = nc.dram_tensor("x8", [512, 2, NL], f8, kind="ExternalInput").ap()
    wqk8_d = nc.dram_tensor("wqk8", [512, 2, 2048], f8, kind="ExternalInput").ap()
    wv8_d = nc.dram_tensor("wv8", [512, 2, 1024], f8, kind="ExternalInput").ap()
    wp8_d = nc.dram_tensor("wp8", [512, 2, 1024], f8, kind="ExternalInput").ap()
    xr_d = nc.dram_tensor("xr", [C, NL], bf, kind="ExternalInput").ap()
    bias_d = nc.dram_tensor("bias", [128, 8], f32, kind="ExternalInput").ap()
    tmpv_d = nc.dram_tensor("tmpv", [128, 8], f32, kind="ExternalInput").ap()
    yT_d = nc.dram_tensor("yT", [C, NL], f32, kind="ExternalOutput").ap()

    with tile.TileContext(nc) as tc:
        with (
            tc.tile_pool(name="const", bufs=1) as constp,
            tc.tile_pool(name="wgt", bufs=1) as wgtp,
            tc.tile_pool(name="xs", bufs=1) as xsp,
            tc.tile_pool(name="qk", bufs=1) as qkp,
            tc.tile_pool(name="vo", bufs=1) as vop,
            tc.tile_pool(name="wrk", bufs=1) as wrk,
            tc.tile_pool(name="ps1", bufs=1, space="PSUM") as ps1,
            tc.tile_pool(name="dram", bufs=1, space="DRAM") as dramp,
        ):
            # ---------------- constants ----------------
            ident = constp.tile([128, 128], f32, name="ident")
            make_identity(nc, ident[:])
            identb = constp.tile([128, 128], bf, name="identb")
            nc.gpsimd.tensor_copy(identb[:], ident[:])
            ident8 = constp.tile([128, 128], f8, name="ident8")
            nc.gpsimd.tensor_copy(ident8[:], ident[:])
            bias_sb = constp.tile([128, 8], f32, name="bias_sb")
            nc.sync.dma_start(bias_sb[:], bias_d[:])
            tmpv_sb = constp.tile([128, 8], f32, name="tmpv_sb")
            nc.sync.dma_start(tmpv_sb[:], tmpv_d[:])
            ones8 = constp.tile([128, 2, 128], f8, name="ones8")
            nc.vector.memset(ones8[:], 1.0)

            # ---------------- bulk input DMA (all SBUF-resident) ---------
            x8 = []
            for t in range(4):
                xt = xsp.tile([128, 2, NL], f8, tag=f"x{t}", name=f"x8_{t}")
                nc.sync.dma_start(xt[:], x8_d[t * 128:(t + 1) * 128])
                x8.append(xt)
            wqk8 = []
            for t in range(4):
                wt = wgtp.tile([128, 2, 2048], f8, tag=f"wqk{t}", name=f"wqk8_{t}")
                nc.sync.dma_start(wt[:], wqk8_d[t * 128:(t + 1) * 128])
                wqk8.append(wt)
            wv8 = []
            for t in range(4):
                wt = wgtp.tile([128, 2, 1024], f8, tag=f"wv{t}", name=f"wv8_{t}")
                nc.sync.dma_start(wt[:], wv8_d[t * 128:(t + 1) * 128])
                wv8.append(wt)
            wp8 = []
            for t in range(4):
                wt = wgtp.tile([128, 2, 1024], f8, tag=f"wp{t}", name=f"wp8_{t}")
                nc.sync.dma_start(wt[:], wp8_d[t * 128:(t + 1) * 128])
                wp8.append(wt)

            P = ["pA", "pB", "pC", "pD"]  # 4 x [128,1024] = 8 banks

            # ---------------- A1a: q,k -> fp8 (+ squares) ----------------
            # qk8[u][p, sl, 0:1024]=q / [1024:2048]=k for token tile 2u+sl;
            # sqk8 = (q/64)^2 etc. (true, unscaled squares; fp8-safe range)
            qk8, sqk8 = [], []
            for u in range(8):
                qt = qkp.tile([128, 2, 2048], f8, tag=f"qk{u}", name=f"qk8_{u}")
                st = qkp.tile([128, 2, 2048], f8, tag=f"sqk{u}", name=f"sqk8_{u}")
                qk8.append(qt)
                sqk8.append(st)
            for u in range(8):
                for sl in range(2):
                    s = 2 * u + sl
                    q_ps = ps1.tile([128, 1024], f32, tag=P[2 * sl], name="q_ps")
                    k_ps = ps1.tile([128, 1024], f32, tag=P[2 * sl + 1], name="k_ps")
                    for c in range(2):
                        for t in range(4):
                            nc.tensor.matmul(
                                q_ps[:, c * 512:(c + 1) * 512],
                                x8[t][:, :, s * 128:(s + 1) * 128],
                                wqk8[t][:, :, c * 512:(c + 1) * 512],
                                start=(t == 0), stop=(t == 3), perf_mode=DR)
                    for c in range(2):
                        for t in range(4):
                            nc.tensor.matmul(
                                k_ps[:, c * 512:(c + 1) * 512],
                                x8[t][:, :, s * 128:(s + 1) * 128],
                                wqk8[t][:, :, 1024 + c * 512:1024 + (c + 1) * 512],
                                start=(t == 0), stop=(t == 3), perf_mode=DR)
                    nc.vector.tensor_copy(qk8[u][:, sl, 0:1024], q_ps[:])
                    nc.vector.tensor_copy(qk8[u][:, sl, 1024:2048], k_ps[:])
                    nc.scalar.activation(sqk8[u][:, sl, 0:1024], q_ps[:],
                                         Square, scale=1.0 / WS)
                    nc.scalar.activation(sqk8[u][:, sl, 1024:2048], k_ps[:],
                                         Square, scale=1.0 / WS)

            # ---------------- A1b: Grams + sumsq --------------------------
            # stA = heads 0,1 / stB = heads 2,3; Gram'[kc,qc] = sum k'q'
            stA = ps1.tile([128, 1024], f32, tag="pA", name="stA")
            stB = ps1.tile([128, 1024], f32, tag="pB", name="stB")
            # ones stationary is M=128 wide (narrow DR ldweights fails the
            # ISA check); every psum row holds the same channel sums
            ssq_q = ps1.tile([128, 1024], f32, tag="pC", name="ssq_q")
            ssq_k = ps1.tile([128, 1024], f32, tag="pD", name="ssq_k")

            def st_slice(h, m):
                t = stA if h < 2 else stB
                off = (h % 2) * 512 + m * 256
                return t[:, off:off + 256]

            for h in range(H):
                for m in range(2):
                    for u in range(8):
                        nc.tensor.matmul(
                            st_slice(h, m),
                            qk8[u][:, :, 1024 + h * 256 + m * 128:
                                   1024 + h * 256 + (m + 1) * 128],
                            qk8[u][:, :, h * 256:(h + 1) * 256],
                            start=(u == 0), stop=(u == 7), perf_mode=DR)
            for c in range(2):
                for u in range(8):
                    nc.tensor.matmul(ssq_q[:, c * 512:(c + 1) * 512], ones8[:],
                                     sqk8[u][:, :, c * 512:(c + 1) * 512],
                                     start=(u == 0), stop=(u == 7), perf_mode=DR)
                    nc.tensor.matmul(ssq_k[:, c * 512:(c + 1) * 512], ones8[:],
                                     sqk8[u][:, :, 1024 + c * 512:1024 + (c + 1) * 512],
                                     start=(u == 0), stop=(u == 7), perf_mode=DR)

            # f32 SBUF bounces for the collective (DMA cannot read PSUM;
            # bf16 AllReduce measured ~2x slower than f32 - slow path)
            stA_sb = wrk.tile([128, 1024], f32, tag="stA_sb", name="stA_sb")
            stB_sb = wrk.tile([128, 1024], f32, tag="stB_sb", name="stB_sb")
            ssq_sb = wrk.tile([1, 2048], f32, tag="ssq_sb", name="ssq_sb")
            nc.vector.tensor_copy(stA_sb[:], stA[:])
            nc.vector.tensor_copy(stB_sb[:], stB[:])
            nc.vector.tensor_copy(ssq_sb[:, 0:1024], ssq_q[0:1, :])
            nc.vector.tensor_copy(ssq_sb[:, 1024:2048], ssq_k[0:1, :])

            # ---------------- AllReduce over batch-pairs (f32) ------------
            CCN = 128 * 2048 + 2048
            cc_in = dramp.tile([CCN], f32, name="cc_in")
            cc_out = dramp.tile([CCN], f32, name="cc_out")
            nc.sync.dma_start(
                cc_in[0:131072].rearrange("(p f) -> p f", p=128), stA_sb[:])
            nc.sync.dma_start(
                cc_in[131072:262144].rearrange("(p f) -> p f", p=128), stB_sb[:])
            nc.sync.dma_start(
                cc_in[262144:264192].rearrange("(a f) -> a f", a=1), ssq_sb[:])
            nc.gpsimd.collective_compute(
                "AllReduce", ADD,
                replica_groups=[[0, 1], [2, 3], [4, 5], [6, 7]],
                ins=[cc_in.opt()], outs=[cc_out.opt()])
            strA = wrk.tile([128, 1024], f32, tag="stA_sb", name="strA")
            strB = wrk.tile([128, 1024], f32, tag="stB_sb", name="strB")
            nc.sync.dma_start(
                strA[:], cc_out[0:131072].rearrange("(p f) -> p f", p=128))
            nc.sync.dma_start(
                strB[:], cc_out[131072:262144].rearrange("(p f) -> p f", p=128))
            ssred = constp.tile([128, 16], f32, name="ssred")
            nc.sync.dma_start(
                ssred[:],
                cc_out[262144:264192].rearrange("(j p) -> p j", p=128))

            def str_slice(h, m):
                t = strA if h < 2 else strB
                off = (h % 2) * 512 + m * 256
                return t[:, off:off + 256]

            # ---------------- A2: v (overlaps the collective) -------------
            # v8[h][p, i, tok] = v'[h*256 + i*128 + p, tok]  (fp8, x64)
            v8 = []
            for h in range(H):
                vt = vop.tile([128, 2, NL], f8, tag=f"v{h}", name=f"v8_{h}")
                v8.append(vt)
            for cv in range(8):
                h, i = cv // 2, cv % 2
                vp0 = ps1.tile([128, 1024], f32, tag=P[2 * (cv % 2)], name="vp0")
                vp1 = ps1.tile([128, 1024], f32, tag=P[2 * (cv % 2) + 1], name="vp1")
                for half, vp in ((0, vp0), (1, vp1)):
                    for c in range(2):
                        for t in range(4):
                            nc.tensor.matmul(
                                vp[:, c * 512:(c + 1) * 512],
                                wv8[t][:, :, cv * 128:(cv + 1) * 128],
                                x8[t][:, :, half * 1024 + c * 512:
                                      half * 1024 + (c + 1) * 512],
                                start=(t == 0), stop=(t == 3), perf_mode=DR)
                nc.vector.tensor_copy(v8[h][:, i, 0:1024], vp0[:])
                nc.vector.tensor_copy(v8[h][:, i, 1024:2048], vp1[:])

            # ---------------- phase B: softmax + attn@v -------------------
            # rqk cols 0-7: rq = temp/max(64*||q||,eps); 8-15: rk = 1/(64||k||)
            rqk = constp.tile([128, 16], f32, name="rqk")
            nc.scalar.activation(rqk[:], ssred[:], Sqrt, scale=WS * WS)
            nc.vector.tensor_scalar_max(rqk[:], rqk[:], 1e-9)
            nc.vector.reciprocal(rqk[:], rqk[:])
            nc.vector.tensor_mul(rqk[:, 0:8], rqk[:, 0:8], tmpv_sb[:])

            # out8 reuses x8's SBUF slots (x8 has no readers after A2)
            out8 = []
            for h in range(H):
                ot = xsp.tile([128, 2, NL], f8, tag=f"x{h}", name=f"out8_{h}")
                out8.append(ot)
            # residual rows, resident through phase C
            xrt = []
            for j in range(8):
                xrj = xsp.tile([128, NL], bf, tag=f"xr{j}", name=f"xr_{j}")
                nc.sync.dma_start(xrj[:], xr_d[j * 128:(j + 1) * 128, :])
                xrt.append(xrj)

            for h in range(H):
                # ---- attention for head h (psum tags pA/pB) ----
                sth = wrk.tile([128, 512], bf, tag="sth", bufs=2, name="sth")
                for m in range(2):
                    nc.vector.tensor_scalar_mul(
                        sth[:, m * 256:(m + 1) * 256], str_slice(h, m),
                        rqk[:, 8 + 2 * h + m: 9 + 2 * h + m])
                spm = ps1.tile([128, 512], bf, tag="pA", name="spm")
                for mc in range(2):
                    for md in range(2):
                        nc.tensor.transpose(
                            spm[:, mc * 256 + md * 128: mc * 256 + (md + 1) * 128],
                            sth[:, md * 256 + mc * 128: md * 256 + (mc + 1) * 128],
                            identb[:])
                sft = wrk.tile([128, 512], f32, tag="sft", bufs=2, name="sft")
                for mc in range(2):
                    nc.vector.tensor_scalar_mul(
                        sft[:, mc * 256:(mc + 1) * 256],
                        spm[:, mc * 256:(mc + 1) * 256],
                        rqk[:, 2 * h + mc: 1 + 2 * h + mc])
                negmax = wrk.tile([128, 2], f32, tag="negmax", bufs=2, name="negmax")
                rowsum = wrk.tile([128, 2], f32, tag="rowsum", bufs=2, name="rowsum")
                recip = wrk.tile([128, 2], f32, tag="recip", bufs=2, name="recip")
                esb = wrk.tile([128, 512], bf, tag="esb", bufs=2, name="esb")
                for mc in range(2):
                    nc.vector.reduce_max(negmax[:, mc:mc + 1],
                                         sft[:, mc * 256:(mc + 1) * 256],
                                         axis=AX, negate=True)
                    nc.scalar.activation(esb[:, mc * 256:(mc + 1) * 256],
                                         sft[:, mc * 256:(mc + 1) * 256],
                                         Exp, bias=negmax[:, mc:mc + 1],
                                         accum_out=rowsum[:, mc:mc + 1])
                nc.vector.reciprocal(recip[:], rowsum[:])
                # attn^T [d, (md, c)] via bf16 PE transposes (fp8 transpose
                # needs stride-2 psum writes per the walrus verifier)
                atp = ps1.tile([128, 512], bf, tag="pB", name="atp")
                for md in range(2):
                    for mc in range(2):
                        nc.tensor.transpose(
                            atp[:, md * 256 + mc * 128: md * 256 + (mc + 1) * 128],
                            esb[:, mc * 256 + md * 128: mc * 256 + (md + 1) * 128],
                            identb[:])
                atn8 = wrk.tile([128, 2, 256], f8, tag="atn8", bufs=2, name="atn8")
                nc.vector.tensor_copy(atn8[:, 0, :], atp[:, 0:256])
                nc.vector.tensor_copy(atn8[:, 1, :], atp[:, 256:512])
                # out' = attn^T(unnorm) @ v', row-scaled by 1/rowsum
                for nfh in range(2):
                    opc = ps1.tile([128, 1024], f32, tag="pA", name="opc")
                    opd = ps1.tile([128, 1024], f32, tag="pB", name="opd")
                    for mc, op in ((0, opc), (1, opd)):
                        for nf2 in range(2):
                            nc.tensor.matmul(
                                op[:, nf2 * 512:(nf2 + 1) * 512],
                                atn8[:, :, mc * 128:(mc + 1) * 128],
                                v8[h][:, :, nfh * 1024 + nf2 * 512:
                                      nfh * 1024 + (nf2 + 1) * 512],
                                start=True, stop=True, perf_mode=DR)
                    for mc, op in ((0, opc), (1, opd)):
                        nc.vector.tensor_scalar_mul(
                            out8[h][:, mc, nfh * 1024:(nfh + 1) * 1024],
                            op[:], recip[:, mc:mc + 1])

                # ---- projection for token quarter q=h (psum tags pC/pD) ----
                # y tokens h*512..: proj of head h's out; bias, residual, store
                for jp in range(4):
                    pq = ps1.tile([128, 1024], f32, tag=P[2 + jp % 2], name="pq")
                    for jj in range(2):
                        j = 2 * jp + jj
                        for u in range(4):
                            nc.tensor.matmul(
                                pq[:, jj * 512:(jj + 1) * 512],
                                wp8[u][:, :, j * 128:(j + 1) * 128],
                                out8[h][:, :, u::4],
                                start=(u == 0), stop=(u == 3), perf_mode=DR)
                    for jj in range(2):
                        j = 2 * jp + jj
                        yq = wrk.tile([128, 512], f32, tag=f"yq{j % 4}", bufs=2,
                                      name=f"yq_{h}_{j}")
                        nc.scalar.activation(yq[:], pq[:, jj * 512:(jj + 1) * 512],
                                             Ident, bias=bias_sb[:, j:j + 1],
                                             scale=1.0 / (WS * WS))
                        nc.vector.tensor_add(
                            yq[:], yq[:], xrt[j][:, h * 512:(h + 1) * 512])
                        nc.sync.dma_start(
                            yT_d[j * 128:(j + 1) * 128, h * 512:(h + 1) * 512],
                            yq[:])

    nc.compile()
    return nc


def _get_nc():
    if "nc" not in _CACHE:
        _CACHE["nc"] = _build()
    return _CACHE["nc"]


def _dr_pack(a):
    """[C, F] channel-major -> DoubleRow [512, 2, F]: row t*128+p carries
    channels (256t+p, 256t+128+p) group-major."""
    Cc, F = a.shape
    return np.ascontiguousarray(
        a.reshape(Cc // 256, 2, 128, F).transpose(0, 2, 1, 3).reshape(
            Cc // 2, 2, F))


def _out_rows(half):
    # torch transpose+reshape scramble: this core's y rows
    return np.concatenate(
        [h * 1024 + half * 512 + np.arange(512) for h in range(H)])


def _make_in_maps(x, Wqkv, Wproj, bproj, temperature):
    x = np.ascontiguousarray(np.asarray(x, dtype=np.float32))
    Wqkv = np.asarray(Wqkv, dtype=np.float32)
    Wproj = np.asarray(Wproj, dtype=np.float32)
    bproj = np.asarray(bproj, dtype=np.float32).reshape(C)
    temp = np.asarray(temperature, dtype=np.float32).reshape(H)

    WqkvT = np.ascontiguousarray(Wqkv.T) * WS     # [C, 3C], prescaled
    wqk8 = _dr_pack(WqkvT[:, :2 * C]).astype(F8)
    wv8 = _dr_pack(WqkvT[:, 2 * C:]).astype(F8)
    wp8 = _dr_pack(np.ascontiguousarray(Wproj.T) * WS).astype(F8)
    bias2d = np.ascontiguousarray(bproj.reshape(8, 128).T)
    tmpv2d = np.ascontiguousarray(np.repeat(temp, HD).reshape(8, 128).T)

    in_maps = []
    for core in range(NCORES):
        b, half = core // 2, core % 2
        xT = np.ascontiguousarray(x[b, half * NL:(half + 1) * NL, :].T)
        x8 = _dr_pack(xT).astype(F8)
        xr = np.ascontiguousarray(x[b, _out_rows(half), :].T).astype(BF16)
        in_maps.append(dict(x8=x8, xr=xr, wqk8=wqk8, wv8=wv8, wp8=wp8,
                            bias=bias2d, tmpv=tmpv2d))
    return in_maps


def _run(in_maps, trace=False, **kw):
    from concourse.bass_utils import run_bass_kernel_spmd

    nc = _get_nc()
    return run_bass_kernel_spmd(nc, in_maps, core_ids=list(range(NCORES)),
                                trace=trace, **kw)


def kernel(x, Wqkv, Wproj, bproj, temperature):
    res = _run(_make_in_maps(x, Wqkv, Wproj, bproj, temperature))
    y = np.empty((B, N, C), dtype=np.float32)
    for core in range(NCORES):
        b, half = core // 2, core % 2
        y[b, _out_rows(half), :] = res.results[core]["yT"].T
    return y
